# revision 1
# baseline (speedup 1.0000x reference)
"""Trainium2 Bass kernel for nn_NormalizedDistanceLoss.

Math: for x in R^{N x D}, with sq_i = ||x_i||^2, the strict-upper-triangle
sum of pairwise squared distances collapses algebraically:

    sum_{i<j} (sq_i + sq_j - 2 x_i.x_j) = N * S - ||s||^2

where S = sum_i sq_i and s = sum_i x_i (column sums).  So the loss

    loss = sum_masked_dist / (sqrt(max_i sq_i) * N(N-1)/2)

needs only one pass over x: per-row squared norms (for S and the max)
and column sums (for s).  Each of the 8 cores reduces its 1024-row block;
the host combines tiny per-core partials (a few KB per core).

Per-core device kernel (block = 1024 x 512 f32):
  - SBUF layout (128, 8, 512): partition p holds DRAM rows p*8..p*8+7
    (16KB contiguous per partition).  4 chunked DMAs (2 row-tiles each)
    split across BOTH HWDGE rings (sync + scalar) so transfers overlap.
  - Row squared norms: one fused square+row-sum op per 512-wide tile;
    ACT (Square activation + accum_out) for even tiles, DVE
    (scalar_tensor_tensor + accum_out) for odd tiles.
  - Column sums: DVE adds each tile pair into a bf16 pair tile; the
    otherwise-idle PE contracts the 128 partitions with a ones-vector
    matmul, accumulating all pairs in one PSUM bank.  bf16 pair rounding
    perturbs the final loss at ~1e-8 relative - far below fp32 noise.
"""

import sys

if "/opt/trn_rl_repo" not in sys.path:
    sys.path.insert(0, "/opt/trn_rl_repo")

import numpy as np

import concourse.bass as bass
import concourse.tile as tile
from concourse import bacc, mybir

N = 8192
D = 512
NCORES = 8
ROWS = N // NCORES  # 1024 rows per core
P = 128
T = ROWS // P  # 8 row-tiles of 512
NCHUNKS = 4
TPC = T // NCHUNKS  # row-tiles per DMA chunk (2)

_nc_cache = []


def _build_nc():
    f32 = mybir.dt.float32
    bf16 = mybir.dt.bfloat16
    nc = bacc.Bacc(
        "TRN2",
        target_bir_lowering=False,
        debug=False,
        num_devices=NCORES,
    )
    x_dram = nc.dram_tensor("x_blk", [ROWS, D], f32, kind="ExternalInput")
    rowsq_dram = nc.dram_tensor("rowsq", [P, T], f32, kind="ExternalOutput")
    colsum_dram = nc.dram_tensor("colsum", [1, D], f32, kind="ExternalOutput")

    with tile.TileContext(nc) as tc:
        with (
            tc.tile_pool(name="xpool", bufs=1) as xpool,
            tc.tile_pool(name="scr_a", bufs=2) as scr_a,
            tc.tile_pool(name="scr_b", bufs=2) as scr_b,
            tc.tile_pool(name="pairs", bufs=4) as pairs,
            tc.tile_pool(name="stats", bufs=1) as stats,
            tc.tile_pool(name="psum", bufs=1, space=bass.MemorySpace.PSUM) as psum_pool,
        ):
            X = xpool.tile([P, T, D], f32)
            # partition p <- DRAM rows p*T .. p*T+T-1 (contiguous 16KB)
            x_r = x_dram[:].rearrange("(p t) d -> p t d", p=P)

            rowsq = stats.tile([P, T], f32)
            ps = psum_pool.tile([1, D], f32)
            onesb = nc.const_aps.tensor(1.0, [P, 1], bf16)

            # 4 chunks of 2 row-tiles alternating between the two HWDGE
            # rings so two transfers are in flight and each chunk's
            # completion semaphore gates only its own tiles' compute.
            for c in range(NCHUNKS):
                sl = slice(c * TPC, (c + 1) * TPC)
                eng = nc.scalar if c % 2 == 0 else nc.sync
                eng.dma_start(X[:, sl, :], x_r[:, sl, :])

            def act_square(t, col):
                xsq_a = scr_a.tile([P, D], f32, tag="xsq_a")
                nc.scalar.activation(
                    xsq_a[:],
                    X[:, t, :],
                    mybir.ActivationFunctionType.Square,
                    accum_out=rowsq[:, col : col + 1],
                )

            def stt_square(eng, t, col, tag, pool):
                xsq = pool.tile([P, D], f32, tag=tag)
                eng.scalar_tensor_tensor(
                    out=xsq[:],
                    in0=X[:, t, :],
                    scalar=1.0,
                    in1=X[:, t, :],
                    op0=mybir.AluOpType.mult,
                    op1=mybir.AluOpType.mult,
                    accum_out=rowsq[:, col : col + 1],
                )

            def pair_mm(c, start, stop):
                pair = pairs.tile([P, D], bf16, tag="pair")
                nc.vector.tensor_add(pair[:], X[:, 2 * c, :], X[:, 2 * c + 1, :])
                nc.tensor.matmul(ps[:], onesb, pair[:], start=start, stop=stop)

            # DVE runs all four pairs as their chunks land (deferred squares
            # queue behind them) so the PSUM accumulation finishes as early
            # as possible; ACT carries five squares plus the PSUM copy.
            pair_mm(0, True, False)
            stt_square(nc.vector, 1, 4, "xsq_b", scr_b)
            act_square(0, 0)
            act_square(2, 1)
            pair_mm(1, False, False)
            pair_mm(2, False, False)
            pair_mm(3, False, True)
            stt_square(nc.vector, 3, 5, "xsq_b", scr_b)
            act_square(4, 2)
            act_square(6, 3)
            act_square(5, 6)
            stt_square(nc.vector, 7, 7, "xsq_b", scr_b)

            colsum = stats.tile([1, D], f32)
            nc.scalar.copy(colsum[:], ps[:])

            nc.sync.dma_start(rowsq_dram[:], rowsq[:])
            nc.scalar.dma_start(colsum_dram[:], colsum[:])

    nc.compile()
    return nc


def get_nc():
    if not _nc_cache:
        _nc_cache.append(_build_nc())
    return _nc_cache[0]


def combine_partials(rowsq_parts, colsum_parts):
    """rowsq_parts: per-core (P, T//2) row-squared-norm arrays; colsum_parts:
    per-core (1, D) column sums -> scalar loss.  Row order is irrelevant
    for sum/max, so no reindexing is needed."""
    S = 0.0
    maxsq = -np.inf
    for r in rowsq_parts:
        S += r.sum(dtype=np.float64)
        maxsq = max(maxsq, float(r.max()))
    s = np.zeros(D, dtype=np.float64)
    for cs in colsum_parts:
        s += cs.reshape(-1).astype(np.float64)
    count = N * (N - 1) // 2
    loss = (N * S - s @ s) / (np.sqrt(maxsq) * count)
    return np.float32(loss)


def kernel(x):
    from concourse.bass_utils import run_bass_kernel_spmd

    x = np.ascontiguousarray(np.asarray(x), dtype=np.float32)
    assert x.shape == (N, D), x.shape
    nc = get_nc()
    in_maps = [{"x_blk": x[c * ROWS : (c + 1) * ROWS]} for c in range(NCORES)]
    res = run_bass_kernel_spmd(nc, in_maps, list(range(NCORES)))
    rowsq_parts = [r["rowsq"] for r in res.results]
    colsum_parts = [r["colsum"] for r in res.results]
    return combine_partials(rowsq_parts, colsum_parts)

